# revision 1
# baseline (speedup 1.0000x reference)
import sys
from contextlib import ExitStack

import numpy as np

sys.path.insert(0, "/opt/trn_rl_repo")

import concourse.bass as bass
import concourse.tile as tile
from concourse import bacc, mybir
from concourse.bass_utils import run_bass_kernel_spmd

# Problem constants (hardcoded per harness contract)
N = 10000
D_IN = 12
E = N * D_IN            # 120000 edges
T = E * D_IN            # 1440000 triplets
K_R = 16
K_A = 8
HID = 64
OUT_D = 32
IN_DIM = 2 * K_R + K_A  # 40
GAMMA = 8.0             # same gamma for radial and angular RBFs
EPS = 1e-8
POISON = 1e9            # drives all RBF features to exp(-huge) = 0

NCORES = 8
TD = T // NCORES        # 180000 triplets per core
ED = E // NCORES        # 15000 edges per core
TT = 504                # triplets per tile = 42 edges * 12

F32 = mybir.dt.float32

_PROG = None
LAST_RESULTS = None
LAST_RUN_S = None


def _build_program():
    nc = bacc.Bacc(
        "TRN2", target_bir_lowering=False, debug=False, num_devices=NCORES
    )
    X = nc.dram_tensor("x", [IN_DIM, TD], F32, kind="ExternalInput").ap()
    W1 = nc.dram_tensor("w1", [IN_DIM, HID], F32, kind="ExternalInput").ap()
    W2 = nc.dram_tensor("w2", [HID, OUT_D], F32, kind="ExternalInput").ap()
    B1 = nc.dram_tensor("b1", [HID, 1], F32, kind="ExternalInput").ap()
    Y = nc.dram_tensor("y", [OUT_D, ED], F32, kind="ExternalOutput").ap()

    with tile.TileContext(nc) as tc, ExitStack() as ctx:
        consts = ctx.enter_context(tc.tile_pool(name="consts", bufs=1))
        inp = ctx.enter_context(tc.tile_pool(name="inp", bufs=4))
        mid = ctx.enter_context(tc.tile_pool(name="mid", bufs=3))
        hp = ctx.enter_context(tc.tile_pool(name="hp", bufs=3))
        ps1 = ctx.enter_context(
            tc.tile_pool(name="ps1", bufs=2, space=bass.MemorySpace.PSUM)
        )
        ps2 = ctx.enter_context(
            tc.tile_pool(name="ps2", bufs=2, space=bass.MemorySpace.PSUM)
        )

        w1t = consts.tile([IN_DIM, HID], F32)
        nc.gpsimd.dma_start(w1t[:], W1[:])
        w2t = consts.tile([HID, OUT_D], F32)
        nc.gpsimd.dma_start(w2t[:], W2[:])
        b1t = consts.tile([HID, 1], F32)
        nc.gpsimd.dma_start(b1t[:], B1[:])
        out_sb = consts.tile([OUT_D, ED], F32)

        ntiles = (TD + TT - 1) // TT
        for i in range(ntiles):
            t0 = i * TT
            tt = min(TT, TD - t0)
            g = tt // D_IN
            e0 = t0 // D_IN

            xt = inp.tile([IN_DIM, tt], F32)
            nc.gpsimd.dma_start(xt[:], X[:, t0 : t0 + tt])

            sq = mid.tile([IN_DIM, tt], F32)
            nc.vector.tensor_mul(sq[:], xt[:], xt[:])

            ft = mid.tile([IN_DIM, tt], F32)
            nc.scalar.activation(
                ft[:], sq[:], mybir.ActivationFunctionType.Exp, scale=-GAMMA
            )

            p1 = ps1.tile([HID, tt], F32)
            nc.tensor.matmul(p1[:], w1t[:], ft[:])

            h = hp.tile([HID, tt], F32)
            nc.scalar.activation(
                h[:], p1[:], mybir.ActivationFunctionType.Silu, bias=b1t[:]
            )

            p2 = ps2.tile([OUT_D, tt], F32)
            nc.tensor.matmul(p2[:], w2t[:], h[:])

            nc.vector.tensor_reduce(
                out_sb[:, e0 : e0 + g],
                p2[:].rearrange("p (g s) -> p g s", s=D_IN),
                axis=mybir.AxisListType.X,
                op=mybir.AluOpType.add,
            )

        nc.gpsimd.dma_start(Y[:], out_sb[:])

    nc.compile()
    return nc


def _get_program():
    global _PROG
    if _PROG is None:
        _PROG = _build_program()
    return _PROG


def _numpy_fallback(pos, W1, b1, W2, b2, rc, ac, e_e, i_e, j_e, k_e):
    rij = pos[j_e] - pos[i_e]
    rik = pos[k_e] - pos[i_e]
    dij = np.sqrt((rij * rij).sum(-1))
    dik = np.sqrt((rik * rik).sum(-1))
    cos = np.clip((rij * rik).sum(-1) / (dij * dik + EPS), -1.0, 1.0)
    feat = np.concatenate(
        [
            np.exp(-GAMMA * (dij[:, None] - rc[None, :]) ** 2),
            np.exp(-GAMMA * (dik[:, None] - rc[None, :]) ** 2),
            np.exp(-GAMMA * (cos[:, None] - ac[None, :]) ** 2),
        ],
        axis=-1,
    ).astype(np.float32)
    hpre = feat @ W1 + b1
    h = hpre / (1.0 + np.exp(-hpre))
    emb = h @ W2 + b2
    emb *= (k_e != j_e)[:, None].astype(np.float32)
    out = np.zeros((E, OUT_D), np.float32)
    np.add.at(out, e_e, emb)
    return out


def kernel(**inputs) -> np.ndarray:
    global LAST_RESULTS
    pos = np.asarray(inputs["pos"], np.float32)
    W1 = np.asarray(inputs["W1"], np.float32)
    b1 = np.asarray(inputs["b1"], np.float32)
    W2 = np.asarray(inputs["W2"], np.float32)
    b2 = np.asarray(inputs["b2"], np.float32)
    rc = np.asarray(inputs["r_centers"], np.float32)
    ac = np.asarray(inputs["a_centers"], np.float32)
    e_e = np.asarray(inputs["e_e"])
    i_e = np.asarray(inputs["i_e"])
    j_e = np.asarray(inputs["j_e"])
    k_e = np.asarray(inputs["k_e"])

    structured = np.array_equal(
        e_e, np.repeat(np.arange(E, dtype=np.int64), D_IN).astype(e_e.dtype)
    )
    if not structured:
        return _numpy_fallback(pos, W1, b1, W2, b2, rc, ac, e_e, i_e, j_e, k_e)

    # Per-triplet geometry on host; device handles RBF + MLP + segment sum.
    pi = pos[i_e]
    pj = pos[j_e]
    pk = pos[k_e]
    rij = pj - pi
    rik = pk - pi
    dij = np.sqrt((rij * rij).sum(-1))
    dik = np.sqrt((rik * rik).sum(-1))
    cos = np.clip((rij * rik).sum(-1) / (dij * dik + EPS), -1.0, 1.0)
    mask = k_e != j_e
    dij = np.where(mask, dij, POISON).astype(np.float32)
    dik = np.where(mask, dik, POISON).astype(np.float32)
    cos = np.where(mask, cos, POISON).astype(np.float32)

    X40 = np.empty((IN_DIM, T), np.float32)
    X40[0:K_R] = dij[None, :] - rc[:, None]
    X40[K_R : 2 * K_R] = dik[None, :] - rc[:, None]
    X40[2 * K_R :] = cos[None, :] - ac[:, None]

    b1c = b1.reshape(HID, 1).copy()
    in_maps = []
    for d in range(NCORES):
        in_maps.append(
            {
                "x": np.ascontiguousarray(X40[:, d * TD : (d + 1) * TD]),
                "w1": W1,
                "w2": W2,
                "b1": b1c,
            }
        )

    import time as _time

    global LAST_RUN_S
    _t0 = _time.time()
    res = run_bass_kernel_spmd(_get_program(), in_maps, list(range(NCORES)))
    LAST_RUN_S = _time.time() - _t0
    LAST_RESULTS = res
    outT = np.concatenate([res.results[d]["y"] for d in range(NCORES)], axis=1)
    out = np.ascontiguousarray(outT.T)

    if b2.any():
        cnt = np.bincount(e_e, weights=mask.astype(np.float64), minlength=E)
        out = out + cnt[:, None].astype(np.float32) * b2[None, :]
    return out



# revision 7
# speedup vs baseline: 6.6953x; 6.6953x over previous
import sys
from contextlib import ExitStack

import numpy as np

sys.path.insert(0, "/opt/trn_rl_repo")

import concourse.bass as bass
import concourse.tile as tile
from concourse import bacc, mybir
from concourse.bass_utils import run_bass_kernel_spmd

# Problem constants (hardcoded per harness contract)
N = 10000
D_IN = 12
E = N * D_IN            # 120000 edges
T = E * D_IN            # 1440000 triplets
K_R = 16
K_A = 8
HID = 64
OUT_D = 32
IN_DIM = 2 * K_R + K_A  # 40
GAMMA = 8.0             # same gamma for radial and angular RBFs
EPS = 1e-8
POISON = 30.0           # exp(-8*(30-c)^2) == 0 in f32; fits fp16

NCORES = 8
TD = T // NCORES        # 180000 triplets per core
ED = E // NCORES        # 15000 edges per core
TT = 504                # triplets per tile = 42 edges * 12

F32 = mybir.dt.float32
F16 = mybir.dt.float16

_PROG = None
LAST_RESULTS = None
LAST_RUN_S = None


def _build_program():
    nc = bacc.Bacc(
        "TRN2", target_bir_lowering=False, debug=False, num_devices=NCORES
    )
    # x rows: 0=dij, 1=dik, 2=cos (per triplet, fp16, poisoned where k==j)
    X = nc.dram_tensor("x", [3, TD], F16, kind="ExternalInput").ap()
    # ka[s,f] = 2*g*c_f for s==src(f) else 0; kb[s,f] = -g for s==src(f) else 0
    KA = nc.dram_tensor("ka", [3, IN_DIM], F32, kind="ExternalInput").ap()
    KB = nc.dram_tensor("kb", [3, IN_DIM], F32, kind="ExternalInput").ap()
    # bias40[f] = -g*c_f^2
    BIAS = nc.dram_tensor("bias40", [IN_DIM, 1], F32, kind="ExternalInput").ap()
    W1 = nc.dram_tensor("w1", [IN_DIM, HID], F32, kind="ExternalInput").ap()
    B1 = nc.dram_tensor("b1", [HID, 1], F32, kind="ExternalInput").ap()
    W2 = nc.dram_tensor("w2", [HID, OUT_D], F32, kind="ExternalInput").ap()
    Y = nc.dram_tensor("y", [OUT_D, ED], F16, kind="ExternalOutput").ap()

    with tile.TileContext(nc) as tc, ExitStack() as ctx:
        consts = ctx.enter_context(tc.tile_pool(name="consts", bufs=1))
        inp = ctx.enter_context(tc.tile_pool(name="inp", bufs=4))
        mid = ctx.enter_context(tc.tile_pool(name="mid", bufs=3))
        hp = ctx.enter_context(tc.tile_pool(name="hp", bufs=3))
        ps0 = ctx.enter_context(
            tc.tile_pool(name="ps0", bufs=2, space=bass.MemorySpace.PSUM)
        )
        ps1 = ctx.enter_context(
            tc.tile_pool(name="ps1", bufs=2, space=bass.MemorySpace.PSUM)
        )
        ps2 = ctx.enter_context(
            tc.tile_pool(name="ps2", bufs=2, space=bass.MemorySpace.PSUM)
        )

        kat = consts.tile([3, IN_DIM], F32)
        nc.gpsimd.dma_start(kat[:], KA[:])
        kbt = consts.tile([3, IN_DIM], F32)
        nc.gpsimd.dma_start(kbt[:], KB[:])
        biast = consts.tile([IN_DIM, 1], F32)
        nc.gpsimd.dma_start(biast[:], BIAS[:])
        w1t = consts.tile([IN_DIM, HID], F32)
        nc.gpsimd.dma_start(w1t[:], W1[:])
        b1t = consts.tile([HID, 1], F32)
        nc.gpsimd.dma_start(b1t[:], B1[:])
        w2t = consts.tile([HID, OUT_D], F32)
        nc.gpsimd.dma_start(w2t[:], W2[:])
        out_sb = consts.tile([OUT_D, ED], F32)
        out16 = consts.tile([OUT_D, ED], F16)

        ntiles = (TD + TT - 1) // TT
        for i in range(ntiles):
            t0 = i * TT
            tt = min(TT, TD - t0)
            g = tt // D_IN
            e0 = t0 // D_IN

            xt = inp.tile([3, tt], F16)
            nc.gpsimd.dma_start(xt[:], X[:, t0 : t0 + tt])

            xf = mid.tile([3, tt], F32)
            nc.vector.tensor_copy(xf[:], xt[:])
            sq = mid.tile([3, tt], F32)
            nc.vector.tensor_mul(sq[:], xt[:], xt[:])

            p0 = ps0.tile([IN_DIM, tt], F32)
            nc.tensor.matmul(p0[:], kat[:], xf[:], start=True, stop=False)
            nc.tensor.matmul(p0[:], kbt[:], sq[:], start=False, stop=True)

            ft = mid.tile([IN_DIM, tt], F32)
            nc.scalar.activation(
                ft[:], p0[:], mybir.ActivationFunctionType.Exp, bias=biast[:]
            )

            p1 = ps1.tile([HID, tt], F32)
            nc.tensor.matmul(p1[:], w1t[:], ft[:])

            h = hp.tile([HID, tt], F32)
            nc.scalar.activation(
                h[:], p1[:], mybir.ActivationFunctionType.Silu, bias=b1t[:]
            )

            p2 = ps2.tile([OUT_D, tt], F32)
            nc.tensor.matmul(p2[:], w2t[:], h[:])

            nc.vector.tensor_reduce(
                out_sb[:, e0 : e0 + g],
                p2[:].rearrange("p (g s) -> p g s", s=D_IN),
                axis=mybir.AxisListType.X,
                op=mybir.AluOpType.add,
            )

        nc.scalar.copy(out16[:], out_sb[:])
        nc.gpsimd.dma_start(Y[:], out16[:])

    nc.compile()
    return nc


def _get_program():
    global _PROG
    if _PROG is None:
        _PROG = _build_program()
    return _PROG


def _numpy_fallback(pos, W1, b1, W2, b2, rc, ac, e_e, i_e, j_e, k_e):
    rij = pos[j_e] - pos[i_e]
    rik = pos[k_e] - pos[i_e]
    dij = np.sqrt((rij * rij).sum(-1))
    dik = np.sqrt((rik * rik).sum(-1))
    cos = np.clip((rij * rik).sum(-1) / (dij * dik + EPS), -1.0, 1.0)
    feat = np.concatenate(
        [
            np.exp(-GAMMA * (dij[:, None] - rc[None, :]) ** 2),
            np.exp(-GAMMA * (dik[:, None] - rc[None, :]) ** 2),
            np.exp(-GAMMA * (cos[:, None] - ac[None, :]) ** 2),
        ],
        axis=-1,
    ).astype(np.float32)
    hpre = feat @ W1 + b1
    h = hpre / (1.0 + np.exp(-hpre))
    emb = h @ W2 + b2
    emb *= (k_e != j_e)[:, None].astype(np.float32)
    out = np.zeros((E, OUT_D), np.float32)
    np.add.at(out, e_e, emb)
    return out


def kernel(**inputs) -> np.ndarray:
    global LAST_RESULTS
    pos = np.asarray(inputs["pos"], np.float32)
    W1 = np.asarray(inputs["W1"], np.float32)
    b1 = np.asarray(inputs["b1"], np.float32)
    W2 = np.asarray(inputs["W2"], np.float32)
    b2 = np.asarray(inputs["b2"], np.float32)
    rc = np.asarray(inputs["r_centers"], np.float32)
    ac = np.asarray(inputs["a_centers"], np.float32)
    e_e = np.asarray(inputs["e_e"])
    i_e = np.asarray(inputs["i_e"])
    j_e = np.asarray(inputs["j_e"])
    k_e = np.asarray(inputs["k_e"])

    structured = np.array_equal(
        e_e, np.repeat(np.arange(E, dtype=np.int64), D_IN).astype(e_e.dtype)
    )
    if not structured:
        return _numpy_fallback(pos, W1, b1, W2, b2, rc, ac, e_e, i_e, j_e, k_e)

    # Per-triplet geometry on host; device handles RBF + MLP + segment sum.
    pi = pos[i_e]
    pj = pos[j_e]
    pk = pos[k_e]
    rij = pj - pi
    rik = pk - pi
    dij = np.sqrt((rij * rij).sum(-1))
    dik = np.sqrt((rik * rik).sum(-1))
    cos = np.clip((rij * rik).sum(-1) / (dij * dik + EPS), -1.0, 1.0)
    mask = k_e != j_e

    x3 = np.empty((3, T), np.float16)
    x3[0] = np.where(mask, dij, POISON)
    x3[1] = np.where(mask, dik, POISON)
    x3[2] = np.where(mask, cos, POISON)

    # Feature f <- source row s(f), center c_f:
    #   exp(-g*(x-c)^2) = exp(-g*x^2 + 2*g*c*x - g*c^2)
    cf = np.concatenate([rc, rc, ac]).astype(np.float32)         # [40]
    src = np.repeat(np.arange(3), [K_R, K_R, K_A])               # [40]
    ka = np.zeros((3, IN_DIM), np.float32)
    ka[src, np.arange(IN_DIM)] = 2.0 * GAMMA * cf
    kb = np.zeros((3, IN_DIM), np.float32)
    kb[src, np.arange(IN_DIM)] = -GAMMA
    bias40 = (-GAMMA * cf * cf).reshape(IN_DIM, 1)
    b1c = b1.reshape(HID, 1).copy()

    in_maps = []
    for d in range(NCORES):
        in_maps.append(
            {
                "x": np.ascontiguousarray(x3[:, d * TD : (d + 1) * TD]),
                "ka": ka,
                "kb": kb,
                "bias40": bias40,
                "w1": W1,
                "b1": b1c,
                "w2": W2,
            }
        )

    import time as _time

    global LAST_RUN_S
    _t0 = _time.time()
    res = run_bass_kernel_spmd(_get_program(), in_maps, list(range(NCORES)))
    LAST_RUN_S = _time.time() - _t0
    LAST_RESULTS = res
    outT = np.concatenate([res.results[d]["y"] for d in range(NCORES)], axis=1)
    out = np.ascontiguousarray(outT.T.astype(np.float32))

    if b2.any():
        cnt = np.bincount(e_e, weights=mask.astype(np.float64), minlength=E)
        out = out + cnt[:, None].astype(np.float32) * b2[None, :]
    return out


# revision 11
# speedup vs baseline: 11.8446x; 1.7691x over previous
import sys
from contextlib import ExitStack

import numpy as np

sys.path.insert(0, "/opt/trn_rl_repo")

import jax

# Persistent compilation cache: warm calls skip the per-call NEFF/walrus
# recompile inside the neuronx_cc hook (the executable is cached on disk
# keyed by HLO, which is identical across calls).
try:
    jax.config.update("jax_compilation_cache_dir", "/tmp/bass_jax_cache")
    jax.config.update("jax_persistent_cache_min_compile_time_secs", 0.0)
    jax.config.update("jax_persistent_cache_min_entry_size_bytes", 0)
except Exception:
    pass

import concourse.bass as bass
import concourse.tile as tile
from concourse import bacc, mybir
from concourse.bass_utils import run_bass_kernel_spmd

# Problem constants (hardcoded per harness contract)
N = 10000
D_IN = 12
E = N * D_IN            # 120000 edges
T = E * D_IN            # 1440000 triplets
K_R = 16
K_A = 8
HID = 64
OUT_D = 32
IN_DIM = 2 * K_R + K_A  # 40
GAMMA = 8.0             # same gamma for radial and angular RBFs
EPS = 1e-8
POISON = 30.0           # exp(-8*(30-c)^2) == 0 in f32; fits fp16

NCORES = 8
TD = T // NCORES        # 180000 triplets per core
ED = E // NCORES        # 15000 edges per core
TT = 504                # triplets per tile = 42 edges * 12

F32 = mybir.dt.float32
F16 = mybir.dt.float16

_PROG = None
LAST_RESULTS = None
LAST_RUN_S = None


def _build_program():
    nc = bacc.Bacc(
        "TRN2", target_bir_lowering=False, debug=False, num_devices=NCORES
    )
    # x rows: 0=dij, 1=dik, 2=cos (per triplet, fp16, poisoned where k==j)
    X = nc.dram_tensor("x", [3, TD], F16, kind="ExternalInput").ap()
    # ka[s,f] = 2*g*c_f for s==src(f) else 0; kb[s,f] = -g for s==src(f) else 0
    KA = nc.dram_tensor("ka", [3, IN_DIM], F32, kind="ExternalInput").ap()
    KB = nc.dram_tensor("kb", [3, IN_DIM], F32, kind="ExternalInput").ap()
    # bias40[f] = -g*c_f^2
    BIAS = nc.dram_tensor("bias40", [IN_DIM, 1], F32, kind="ExternalInput").ap()
    W1 = nc.dram_tensor("w1", [IN_DIM, HID], F32, kind="ExternalInput").ap()
    B1 = nc.dram_tensor("b1", [HID, 1], F32, kind="ExternalInput").ap()
    W2 = nc.dram_tensor("w2", [HID, OUT_D], F32, kind="ExternalInput").ap()
    Y = nc.dram_tensor("y", [OUT_D, ED], F16, kind="ExternalOutput").ap()

    with tile.TileContext(nc) as tc, ExitStack() as ctx:
        consts = ctx.enter_context(tc.tile_pool(name="consts", bufs=1))
        inp = ctx.enter_context(tc.tile_pool(name="inp", bufs=4))
        mid = ctx.enter_context(tc.tile_pool(name="mid", bufs=3))
        hp = ctx.enter_context(tc.tile_pool(name="hp", bufs=3))
        ps0 = ctx.enter_context(
            tc.tile_pool(name="ps0", bufs=2, space=bass.MemorySpace.PSUM)
        )
        ps1 = ctx.enter_context(
            tc.tile_pool(name="ps1", bufs=2, space=bass.MemorySpace.PSUM)
        )
        ps2 = ctx.enter_context(
            tc.tile_pool(name="ps2", bufs=2, space=bass.MemorySpace.PSUM)
        )

        kat = consts.tile([3, IN_DIM], F32)
        nc.gpsimd.dma_start(kat[:], KA[:])
        kbt = consts.tile([3, IN_DIM], F32)
        nc.gpsimd.dma_start(kbt[:], KB[:])
        biast = consts.tile([IN_DIM, 1], F32)
        nc.gpsimd.dma_start(biast[:], BIAS[:])
        w1t = consts.tile([IN_DIM, HID], F32)
        nc.gpsimd.dma_start(w1t[:], W1[:])
        b1t = consts.tile([HID, 1], F32)
        nc.gpsimd.dma_start(b1t[:], B1[:])
        w2t = consts.tile([HID, OUT_D], F32)
        nc.gpsimd.dma_start(w2t[:], W2[:])
        out_sb = consts.tile([OUT_D, ED], F32)
        out16 = consts.tile([OUT_D, ED], F16)

        ntiles = (TD + TT - 1) // TT
        for i in range(ntiles):
            t0 = i * TT
            tt = min(TT, TD - t0)
            g = tt // D_IN
            e0 = t0 // D_IN

            xt = inp.tile([3, tt], F16)
            nc.gpsimd.dma_start(xt[:], X[:, t0 : t0 + tt])

            xf = mid.tile([3, tt], F32)
            nc.vector.tensor_copy(xf[:], xt[:])
            sq = mid.tile([3, tt], F32)
            nc.vector.tensor_mul(sq[:], xt[:], xt[:])

            p0 = ps0.tile([IN_DIM, tt], F32)
            nc.tensor.matmul(p0[:], kat[:], xf[:], start=True, stop=False)
            nc.tensor.matmul(p0[:], kbt[:], sq[:], start=False, stop=True)

            ft = mid.tile([IN_DIM, tt], F32)
            nc.scalar.activation(
                ft[:], p0[:], mybir.ActivationFunctionType.Exp, bias=biast[:]
            )

            p1 = ps1.tile([HID, tt], F32)
            nc.tensor.matmul(p1[:], w1t[:], ft[:])

            h = hp.tile([HID, tt], F32)
            nc.scalar.activation(
                h[:], p1[:], mybir.ActivationFunctionType.Silu, bias=b1t[:]
            )

            p2 = ps2.tile([OUT_D, tt], F32)
            nc.tensor.matmul(p2[:], w2t[:], h[:])

            nc.vector.tensor_reduce(
                out_sb[:, e0 : e0 + g],
                p2[:].rearrange("p (g s) -> p g s", s=D_IN),
                axis=mybir.AxisListType.X,
                op=mybir.AluOpType.add,
            )

        nc.scalar.copy(out16[:], out_sb[:])
        nc.gpsimd.dma_start(Y[:], out16[:])

    nc.compile()
    return nc


def _get_program():
    global _PROG
    if _PROG is None:
        _PROG = _build_program()
    return _PROG


def _numpy_fallback(pos, W1, b1, W2, b2, rc, ac, e_e, i_e, j_e, k_e):
    rij = pos[j_e] - pos[i_e]
    rik = pos[k_e] - pos[i_e]
    dij = np.sqrt((rij * rij).sum(-1))
    dik = np.sqrt((rik * rik).sum(-1))
    cos = np.clip((rij * rik).sum(-1) / (dij * dik + EPS), -1.0, 1.0)
    feat = np.concatenate(
        [
            np.exp(-GAMMA * (dij[:, None] - rc[None, :]) ** 2),
            np.exp(-GAMMA * (dik[:, None] - rc[None, :]) ** 2),
            np.exp(-GAMMA * (cos[:, None] - ac[None, :]) ** 2),
        ],
        axis=-1,
    ).astype(np.float32)
    hpre = feat @ W1 + b1
    h = hpre / (1.0 + np.exp(-hpre))
    emb = h @ W2 + b2
    emb *= (k_e != j_e)[:, None].astype(np.float32)
    out = np.zeros((E, OUT_D), np.float32)
    np.add.at(out, e_e, emb)
    return out


def kernel(**inputs) -> np.ndarray:
    global LAST_RESULTS
    pos = np.asarray(inputs["pos"], np.float32)
    W1 = np.asarray(inputs["W1"], np.float32)
    b1 = np.asarray(inputs["b1"], np.float32)
    W2 = np.asarray(inputs["W2"], np.float32)
    b2 = np.asarray(inputs["b2"], np.float32)
    rc = np.asarray(inputs["r_centers"], np.float32)
    ac = np.asarray(inputs["a_centers"], np.float32)
    e_e = np.asarray(inputs["e_e"])
    i_e = np.asarray(inputs["i_e"])
    j_e = np.asarray(inputs["j_e"])
    k_e = np.asarray(inputs["k_e"])

    row = i_e[::D_IN].astype(np.int64)          # source node of each edge
    kidx = (row[:, None] * D_IN + np.arange(D_IN)[None, :]).reshape(-1)  # [T]
    structured = (
        np.array_equal(
            e_e, np.repeat(np.arange(E, dtype=np.int64), D_IN).astype(e_e.dtype)
        )
        and np.array_equal(j_e.astype(np.int64), e_e.astype(np.int64) // D_IN)
        and np.array_equal(i_e.astype(np.int64), np.repeat(row, D_IN))
        and np.array_equal(k_e.astype(np.int64), row[kidx])
    )
    if not structured:
        return _numpy_fallback(pos, W1, b1, W2, b2, rc, ac, e_e, i_e, j_e, k_e)

    # Per-edge geometry on host (E values instead of T), then expand to
    # triplets; device handles RBF + MLP + segment sum.
    col = np.repeat(np.arange(N, dtype=np.int64), D_IN)
    dvec = pos[col] - pos[row]                  # [E,3]
    d = np.sqrt((dvec * dvec).sum(-1))          # [E]
    u = dvec / np.maximum(d, 1e-30)[:, None]    # [E,3] unit vectors

    dij = np.repeat(d, D_IN)                    # [T]
    dik = d[kidx]                               # [T]
    # edge kidx points k->i, so rik = pos[k]-pos[i] = -dvec[kidx]
    cos = np.clip(
        -np.einsum("ts,ts->t", np.repeat(u, D_IN, axis=0), u[kidx]), -1.0, 1.0
    )
    mask = k_e != j_e

    x3 = np.empty((3, T), np.float16)
    x3[0] = np.where(mask, dij, POISON)
    x3[1] = np.where(mask, dik, POISON)
    x3[2] = np.where(mask, cos, POISON)

    # Feature f <- source row s(f), center c_f:
    #   exp(-g*(x-c)^2) = exp(-g*x^2 + 2*g*c*x - g*c^2)
    cf = np.concatenate([rc, rc, ac]).astype(np.float32)         # [40]
    src = np.repeat(np.arange(3), [K_R, K_R, K_A])               # [40]
    ka = np.zeros((3, IN_DIM), np.float32)
    ka[src, np.arange(IN_DIM)] = 2.0 * GAMMA * cf
    kb = np.zeros((3, IN_DIM), np.float32)
    kb[src, np.arange(IN_DIM)] = -GAMMA
    bias40 = (-GAMMA * cf * cf).reshape(IN_DIM, 1)
    b1c = b1.reshape(HID, 1).copy()

    in_maps = []
    for d in range(NCORES):
        in_maps.append(
            {
                "x": np.ascontiguousarray(x3[:, d * TD : (d + 1) * TD]),
                "ka": ka,
                "kb": kb,
                "bias40": bias40,
                "w1": W1,
                "b1": b1c,
                "w2": W2,
            }
        )

    import time as _time

    global LAST_RUN_S
    _t0 = _time.time()
    res = run_bass_kernel_spmd(_get_program(), in_maps, list(range(NCORES)))
    LAST_RUN_S = _time.time() - _t0
    LAST_RESULTS = res
    outT = np.concatenate([res.results[d]["y"] for d in range(NCORES)], axis=1)
    out = np.ascontiguousarray(outT.T.astype(np.float32))

    if b2.any():
        cnt = np.bincount(e_e, weights=mask.astype(np.float64), minlength=E)
        out = out + cnt[:, None].astype(np.float32) * b2[None, :]
    return out


# revision 12
# speedup vs baseline: 14.8275x; 1.2518x over previous
import sys
from contextlib import ExitStack

import numpy as np

sys.path.insert(0, "/opt/trn_rl_repo")

import jax

# Persistent compilation cache: warm calls skip the per-call NEFF/walrus
# recompile inside the neuronx_cc hook (the executable is cached on disk
# keyed by HLO, which is identical across calls).
try:
    jax.config.update("jax_compilation_cache_dir", "/tmp/bass_jax_cache")
    jax.config.update("jax_persistent_cache_min_compile_time_secs", 0.0)
    jax.config.update("jax_persistent_cache_min_entry_size_bytes", 0)
except Exception:
    pass

import concourse.bass as bass
import concourse.tile as tile
from concourse import bacc, mybir
from concourse.bass_utils import run_bass_kernel_spmd

# Problem constants (hardcoded per harness contract)
N = 10000
D_IN = 12
E = N * D_IN            # 120000 edges
T = E * D_IN            # 1440000 triplets
K_R = 16
K_A = 8
HID = 64
OUT_D = 32
IN_DIM = 2 * K_R + K_A  # 40
GAMMA = 8.0             # same gamma for radial and angular RBFs
EPS = 1e-8
POISON = 30.0           # exp(-8*(30-c)^2) == 0 in f32; fits fp16

NCORES = 8
TD = T // NCORES        # 180000 triplets per core
ED = E // NCORES        # 15000 edges per core
TT = 504                # triplets per tile = 42 edges * 12

F32 = mybir.dt.float32
F16 = mybir.dt.float16

_PROG = None
LAST_RESULTS = None
LAST_RUN_S = None


def _build_program():
    nc = bacc.Bacc(
        "TRN2", target_bir_lowering=False, debug=False, num_devices=NCORES
    )
    # x rows: 0=dij, 1=dik, 2=cos (per triplet, fp16, poisoned where k==j)
    X = nc.dram_tensor("x", [3, TD], F16, kind="ExternalInput").ap()
    # ka[s,f] = 2*g*c_f for s==src(f) else 0; kb[s,f] = -g for s==src(f) else 0
    KA = nc.dram_tensor("ka", [3, IN_DIM], F32, kind="ExternalInput").ap()
    KB = nc.dram_tensor("kb", [3, IN_DIM], F32, kind="ExternalInput").ap()
    # bias40[f] = -g*c_f^2
    BIAS = nc.dram_tensor("bias40", [IN_DIM, 1], F32, kind="ExternalInput").ap()
    W1 = nc.dram_tensor("w1", [IN_DIM, HID], F32, kind="ExternalInput").ap()
    B1 = nc.dram_tensor("b1", [HID, 1], F32, kind="ExternalInput").ap()
    W2 = nc.dram_tensor("w2", [HID, OUT_D], F32, kind="ExternalInput").ap()
    Y = nc.dram_tensor("y", [OUT_D, ED], F16, kind="ExternalOutput").ap()

    with tile.TileContext(nc) as tc, ExitStack() as ctx:
        consts = ctx.enter_context(tc.tile_pool(name="consts", bufs=1))
        inp = ctx.enter_context(tc.tile_pool(name="inp", bufs=4))
        mid = ctx.enter_context(tc.tile_pool(name="mid", bufs=3))
        hp = ctx.enter_context(tc.tile_pool(name="hp", bufs=3))
        ps0 = ctx.enter_context(
            tc.tile_pool(name="ps0", bufs=2, space=bass.MemorySpace.PSUM)
        )
        ps1 = ctx.enter_context(
            tc.tile_pool(name="ps1", bufs=2, space=bass.MemorySpace.PSUM)
        )
        ps2 = ctx.enter_context(
            tc.tile_pool(name="ps2", bufs=2, space=bass.MemorySpace.PSUM)
        )

        kat = consts.tile([3, IN_DIM], F32)
        nc.gpsimd.dma_start(kat[:], KA[:])
        kbt = consts.tile([3, IN_DIM], F32)
        nc.gpsimd.dma_start(kbt[:], KB[:])
        biast = consts.tile([IN_DIM, 1], F32)
        nc.gpsimd.dma_start(biast[:], BIAS[:])
        w1t = consts.tile([IN_DIM, HID], F32)
        nc.gpsimd.dma_start(w1t[:], W1[:])
        b1t = consts.tile([HID, 1], F32)
        nc.gpsimd.dma_start(b1t[:], B1[:])
        w2t = consts.tile([HID, OUT_D], F32)
        nc.gpsimd.dma_start(w2t[:], W2[:])
        out_sb = consts.tile([OUT_D, ED], F32)
        out16 = consts.tile([OUT_D, ED], F16)

        G = TT // D_IN  # edges per tile

        def emit_tile(t0, e0, tt, g):
            """One tile of `tt` triplets; t0/e0 may be symbolic (hw loop)."""
            xt = inp.tile([3, tt], F16)
            nc.gpsimd.dma_start(xt[:], X[:, bass.ds(t0, tt)])

            xf = mid.tile([3, tt], F32)
            nc.vector.tensor_copy(xf[:], xt[:])
            sq = mid.tile([3, tt], F32)
            nc.vector.tensor_mul(sq[:], xt[:], xt[:])

            p0 = ps0.tile([IN_DIM, tt], F32)
            nc.tensor.matmul(p0[:], kat[:], xf[:], start=True, stop=False)
            nc.tensor.matmul(p0[:], kbt[:], sq[:], start=False, stop=True)

            ft = mid.tile([IN_DIM, tt], F32)
            nc.scalar.activation(
                ft[:], p0[:], mybir.ActivationFunctionType.Exp, bias=biast[:]
            )

            p1 = ps1.tile([HID, tt], F32)
            nc.tensor.matmul(p1[:], w1t[:], ft[:])

            h = hp.tile([HID, tt], F32)
            nc.scalar.activation(
                h[:], p1[:], mybir.ActivationFunctionType.Silu, bias=b1t[:]
            )

            p2 = ps2.tile([OUT_D, tt], F32)
            nc.tensor.matmul(p2[:], w2t[:], h[:])

            nc.vector.tensor_reduce(
                out_sb[:, bass.ds(e0, g)],
                p2[:].rearrange("p (g s) -> p g s", s=D_IN),
                axis=mybir.AxisListType.X,
                op=mybir.AluOpType.add,
            )

        nt_full = TD // TT
        tc.For_i_unrolled(
            0, nt_full, 1, lambda iv: emit_tile(iv * TT, iv * G, TT, G), max_unroll=8
        )
        rem = TD - nt_full * TT
        if rem:
            emit_tile(nt_full * TT, nt_full * G, rem, rem // D_IN)

        nc.scalar.copy(out16[:], out_sb[:])
        nc.gpsimd.dma_start(Y[:], out16[:])

    nc.compile()
    return nc


def _get_program():
    global _PROG
    if _PROG is None:
        _PROG = _build_program()
    return _PROG


def _numpy_fallback(pos, W1, b1, W2, b2, rc, ac, e_e, i_e, j_e, k_e):
    rij = pos[j_e] - pos[i_e]
    rik = pos[k_e] - pos[i_e]
    dij = np.sqrt((rij * rij).sum(-1))
    dik = np.sqrt((rik * rik).sum(-1))
    cos = np.clip((rij * rik).sum(-1) / (dij * dik + EPS), -1.0, 1.0)
    feat = np.concatenate(
        [
            np.exp(-GAMMA * (dij[:, None] - rc[None, :]) ** 2),
            np.exp(-GAMMA * (dik[:, None] - rc[None, :]) ** 2),
            np.exp(-GAMMA * (cos[:, None] - ac[None, :]) ** 2),
        ],
        axis=-1,
    ).astype(np.float32)
    hpre = feat @ W1 + b1
    h = hpre / (1.0 + np.exp(-hpre))
    emb = h @ W2 + b2
    emb *= (k_e != j_e)[:, None].astype(np.float32)
    out = np.zeros((E, OUT_D), np.float32)
    np.add.at(out, e_e, emb)
    return out


def kernel(**inputs) -> np.ndarray:
    global LAST_RESULTS
    pos = np.asarray(inputs["pos"], np.float32)
    W1 = np.asarray(inputs["W1"], np.float32)
    b1 = np.asarray(inputs["b1"], np.float32)
    W2 = np.asarray(inputs["W2"], np.float32)
    b2 = np.asarray(inputs["b2"], np.float32)
    rc = np.asarray(inputs["r_centers"], np.float32)
    ac = np.asarray(inputs["a_centers"], np.float32)
    e_e = np.asarray(inputs["e_e"])
    i_e = np.asarray(inputs["i_e"])
    j_e = np.asarray(inputs["j_e"])
    k_e = np.asarray(inputs["k_e"])

    row = i_e[::D_IN].astype(np.int64)          # source node of each edge
    kidx = (row[:, None] * D_IN + np.arange(D_IN)[None, :]).reshape(-1)  # [T]
    structured = (
        np.array_equal(
            e_e, np.repeat(np.arange(E, dtype=np.int64), D_IN).astype(e_e.dtype)
        )
        and np.array_equal(j_e.astype(np.int64), e_e.astype(np.int64) // D_IN)
        and np.array_equal(i_e.astype(np.int64), np.repeat(row, D_IN))
        and np.array_equal(k_e.astype(np.int64), row[kidx])
    )
    if not structured:
        return _numpy_fallback(pos, W1, b1, W2, b2, rc, ac, e_e, i_e, j_e, k_e)

    # Per-edge geometry on host (E values instead of T), then expand to
    # triplets; device handles RBF + MLP + segment sum.
    col = np.repeat(np.arange(N, dtype=np.int64), D_IN)
    dvec = pos[col] - pos[row]                  # [E,3]
    d = np.sqrt((dvec * dvec).sum(-1))          # [E]
    u = dvec / np.maximum(d, 1e-30)[:, None]    # [E,3] unit vectors

    dij = np.repeat(d, D_IN)                    # [T]
    dik = d[kidx]                               # [T]
    # edge kidx points k->i, so rik = pos[k]-pos[i] = -dvec[kidx]
    cos = np.clip(
        -np.einsum("ts,ts->t", np.repeat(u, D_IN, axis=0), u[kidx]), -1.0, 1.0
    )
    mask = k_e != j_e

    x3 = np.empty((3, T), np.float16)
    x3[0] = np.where(mask, dij, POISON)
    x3[1] = np.where(mask, dik, POISON)
    x3[2] = np.where(mask, cos, POISON)

    # Feature f <- source row s(f), center c_f:
    #   exp(-g*(x-c)^2) = exp(-g*x^2 + 2*g*c*x - g*c^2)
    cf = np.concatenate([rc, rc, ac]).astype(np.float32)         # [40]
    src = np.repeat(np.arange(3), [K_R, K_R, K_A])               # [40]
    ka = np.zeros((3, IN_DIM), np.float32)
    ka[src, np.arange(IN_DIM)] = 2.0 * GAMMA * cf
    kb = np.zeros((3, IN_DIM), np.float32)
    kb[src, np.arange(IN_DIM)] = -GAMMA
    bias40 = (-GAMMA * cf * cf).reshape(IN_DIM, 1)
    b1c = b1.reshape(HID, 1).copy()

    in_maps = []
    for d in range(NCORES):
        in_maps.append(
            {
                "x": np.ascontiguousarray(x3[:, d * TD : (d + 1) * TD]),
                "ka": ka,
                "kb": kb,
                "bias40": bias40,
                "w1": W1,
                "b1": b1c,
                "w2": W2,
            }
        )

    import time as _time

    global LAST_RUN_S
    _t0 = _time.time()
    res = run_bass_kernel_spmd(_get_program(), in_maps, list(range(NCORES)))
    LAST_RUN_S = _time.time() - _t0
    LAST_RESULTS = res
    outT = np.concatenate([res.results[d]["y"] for d in range(NCORES)], axis=1)
    out = np.ascontiguousarray(outT.T.astype(np.float32))

    if b2.any():
        cnt = np.bincount(e_e, weights=mask.astype(np.float64), minlength=E)
        out = out + cnt[:, None].astype(np.float32) * b2[None, :]
    return out


# revision 13
# speedup vs baseline: 15.6036x; 1.0523x over previous
import sys
from contextlib import ExitStack

import numpy as np

sys.path.insert(0, "/opt/trn_rl_repo")

import jax

# Persistent compilation cache: warm calls skip the per-call NEFF/walrus
# recompile inside the neuronx_cc hook (the executable is cached on disk
# keyed by HLO, which is identical across calls).
try:
    jax.config.update("jax_compilation_cache_dir", "/tmp/bass_jax_cache")
    jax.config.update("jax_persistent_cache_min_compile_time_secs", 0.0)
    jax.config.update("jax_persistent_cache_min_entry_size_bytes", 0)
except Exception:
    pass

import concourse.bass as bass
import concourse.tile as tile
from concourse import bacc, mybir
from concourse.bass_utils import run_bass_kernel_spmd

# Problem constants (hardcoded per harness contract)
N = 10000
D_IN = 12
E = N * D_IN            # 120000 edges
T = E * D_IN            # 1440000 triplets
K_R = 16
K_A = 8
HID = 64
OUT_D = 32
IN_DIM = 2 * K_R + K_A  # 40
GAMMA = 8.0             # same gamma for radial and angular RBFs
EPS = 1e-8
POISON = 30.0           # exp(-8*(30-c)^2) == 0 in f32; fits fp16

NCORES = 8
TD = T // NCORES        # 180000 triplets per core
ED = E // NCORES        # 15000 edges per core
TT = 504                # triplets per tile = 42 edges * 12

# params packing offsets (flat f32 tensor)
P_KDA = 0               # [1,40]  2*g*c_f for dij features, else 0
P_KDB = 40              # [1,40]  -g for dij features, else 0
P_KA2 = 80              # [2,40]  2*g*c_f for dik/cos features (row=src-2)
P_KB2 = 160             # [2,40]  -g for dik/cos features
P_BIAS = 240            # [40,1]  -g*c_f^2
P_W1 = 280              # [40,64]
P_B1 = 2840             # [64,1]
P_W2 = 2904             # [64,32]
P_TOT = 4952

F32 = mybir.dt.float32
F16 = mybir.dt.float16

_PROG = None
LAST_RESULTS = None
LAST_RUN_S = None


def _build_program():
    nc = bacc.Bacc(
        "TRN2", target_bir_lowering=False, debug=False, num_devices=NCORES
    )
    # xd: per-edge distance (fp16, unpoisoned); one value per edge, the
    # dij RBF block is per-edge (broadcast over the 12 triplets on device).
    XD = nc.dram_tensor("xd", [1, ED], F16, kind="ExternalInput").ap()
    # xkc rows: 0=dik, 1=cos (per triplet, fp16, poisoned where k==j)
    XKC = nc.dram_tensor("xkc", [2, TD], F16, kind="ExternalInput").ap()
    PRM = nc.dram_tensor("params", [P_TOT], F32, kind="ExternalInput").ap()
    Y = nc.dram_tensor("y", [OUT_D, ED], F16, kind="ExternalOutput").ap()

    with tile.TileContext(nc) as tc, ExitStack() as ctx:
        consts = ctx.enter_context(tc.tile_pool(name="consts", bufs=1))
        inp = ctx.enter_context(tc.tile_pool(name="inp", bufs=4))
        mid = ctx.enter_context(tc.tile_pool(name="mid", bufs=3))
        hp = ctx.enter_context(tc.tile_pool(name="hp", bufs=3))
        ps0 = ctx.enter_context(
            tc.tile_pool(name="ps0", bufs=2, space=bass.MemorySpace.PSUM)
        )
        ps1 = ctx.enter_context(
            tc.tile_pool(name="ps1", bufs=2, space=bass.MemorySpace.PSUM)
        )
        ps2 = ctx.enter_context(
            tc.tile_pool(name="ps2", bufs=2, space=bass.MemorySpace.PSUM)
        )

        kda = consts.tile([1, IN_DIM], F32)
        nc.gpsimd.dma_start(kda[:], PRM[P_KDA : P_KDA + 40].unsqueeze(0))
        kdb = consts.tile([1, IN_DIM], F32)
        nc.gpsimd.dma_start(kdb[:], PRM[P_KDB : P_KDB + 40].unsqueeze(0))
        ka2 = consts.tile([2, IN_DIM], F32)
        nc.gpsimd.dma_start(
            ka2[:], PRM[P_KA2 : P_KA2 + 80].rearrange("(p f) -> p f", p=2)
        )
        kb2 = consts.tile([2, IN_DIM], F32)
        nc.gpsimd.dma_start(
            kb2[:], PRM[P_KB2 : P_KB2 + 80].rearrange("(p f) -> p f", p=2)
        )
        biast = consts.tile([IN_DIM, 1], F32)
        nc.gpsimd.dma_start(
            biast[:], PRM[P_BIAS : P_BIAS + 40].rearrange("(p f) -> p f", p=40)
        )
        w1t = consts.tile([IN_DIM, HID], F32)
        nc.gpsimd.dma_start(
            w1t[:], PRM[P_W1 : P_W1 + 2560].rearrange("(p f) -> p f", p=40)
        )
        b1t = consts.tile([HID, 1], F32)
        nc.gpsimd.dma_start(
            b1t[:], PRM[P_B1 : P_B1 + 64].rearrange("(p f) -> p f", p=64)
        )
        w2t = consts.tile([HID, OUT_D], F32)
        nc.gpsimd.dma_start(
            w2t[:], PRM[P_W2 : P_W2 + 2048].rearrange("(p f) -> p f", p=64)
        )
        out_sb = consts.tile([OUT_D, ED], F32)
        out16 = consts.tile([OUT_D, ED], F16)

        G = TT // D_IN  # edges per tile

        def emit_tile(t0, e0, tt, g):
            """One tile of `tt` triplets / `g` edges; t0/e0 may be symbolic."""
            de = inp.tile([1, g], F16)
            nc.gpsimd.dma_start(de[:], XD[:, bass.ds(e0, g)])
            xt = inp.tile([2, tt], F16)
            nc.gpsimd.dma_start(xt[:], XKC[:, bass.ds(t0, tt)])

            df = mid.tile([1, g], F32)
            nc.vector.tensor_copy(df[:], de[:])
            dsq = mid.tile([1, g], F32)
            nc.vector.tensor_mul(dsq[:], de[:], de[:])
            xf = mid.tile([2, tt], F32)
            nc.vector.tensor_copy(xf[:], xt[:])
            sq = mid.tile([2, tt], F32)
            nc.vector.tensor_mul(sq[:], xt[:], xt[:])

            # per-edge rows broadcast over the 12 triplets of each edge
            df_b = df[:].unsqueeze(2).broadcast_to([1, g, D_IN])
            dsq_b = dsq[:].unsqueeze(2).broadcast_to([1, g, D_IN])

            p0 = ps0.tile([IN_DIM, tt], F32)
            nc.tensor.matmul(p0[:], kda[:], df_b, start=True, stop=False)
            nc.tensor.matmul(p0[:], kdb[:], dsq_b, start=False, stop=False)
            nc.tensor.matmul(p0[:], ka2[:], xf[:], start=False, stop=False)
            nc.tensor.matmul(p0[:], kb2[:], sq[:], start=False, stop=True)

            ft = mid.tile([IN_DIM, tt], F32)
            nc.scalar.activation(
                ft[:], p0[:], mybir.ActivationFunctionType.Exp, bias=biast[:]
            )

            p1 = ps1.tile([HID, tt], F32)
            nc.tensor.matmul(p1[:], w1t[:], ft[:])

            h = hp.tile([HID, tt], F32)
            nc.scalar.activation(
                h[:], p1[:], mybir.ActivationFunctionType.Silu, bias=b1t[:]
            )

            p2 = ps2.tile([OUT_D, tt], F32)
            nc.tensor.matmul(p2[:], w2t[:], h[:])

            nc.vector.tensor_reduce(
                out_sb[:, bass.ds(e0, g)],
                p2[:].rearrange("p (g s) -> p g s", s=D_IN),
                axis=mybir.AxisListType.X,
                op=mybir.AluOpType.add,
            )

        nt_full = TD // TT
        tc.For_i_unrolled(
            0,
            nt_full,
            1,
            lambda iv: emit_tile(iv * TT, iv * (TT // D_IN), TT, TT // D_IN),
            max_unroll=8,
        )
        rem = TD - nt_full * TT
        if rem:
            emit_tile(nt_full * TT, nt_full * G, rem, rem // D_IN)

        nc.scalar.copy(out16[:], out_sb[:])
        nc.gpsimd.dma_start(Y[:], out16[:])

    nc.compile()
    return nc


def _get_program():
    global _PROG
    if _PROG is None:
        _PROG = _build_program()
    return _PROG


def _numpy_fallback(pos, W1, b1, W2, b2, rc, ac, e_e, i_e, j_e, k_e):
    rij = pos[j_e] - pos[i_e]
    rik = pos[k_e] - pos[i_e]
    dij = np.sqrt((rij * rij).sum(-1))
    dik = np.sqrt((rik * rik).sum(-1))
    cos = np.clip((rij * rik).sum(-1) / (dij * dik + EPS), -1.0, 1.0)
    feat = np.concatenate(
        [
            np.exp(-GAMMA * (dij[:, None] - rc[None, :]) ** 2),
            np.exp(-GAMMA * (dik[:, None] - rc[None, :]) ** 2),
            np.exp(-GAMMA * (cos[:, None] - ac[None, :]) ** 2),
        ],
        axis=-1,
    ).astype(np.float32)
    hpre = feat @ W1 + b1
    h = hpre / (1.0 + np.exp(-hpre))
    emb = h @ W2 + b2
    emb *= (k_e != j_e)[:, None].astype(np.float32)
    out = np.zeros((E, OUT_D), np.float32)
    np.add.at(out, e_e, emb)
    return out


def kernel(**inputs) -> np.ndarray:
    global LAST_RESULTS
    pos = np.asarray(inputs["pos"], np.float32)
    W1 = np.asarray(inputs["W1"], np.float32)
    b1 = np.asarray(inputs["b1"], np.float32)
    W2 = np.asarray(inputs["W2"], np.float32)
    b2 = np.asarray(inputs["b2"], np.float32)
    rc = np.asarray(inputs["r_centers"], np.float32)
    ac = np.asarray(inputs["a_centers"], np.float32)
    e_e = np.asarray(inputs["e_e"])
    i_e = np.asarray(inputs["i_e"])
    j_e = np.asarray(inputs["j_e"])
    k_e = np.asarray(inputs["k_e"])

    row = i_e[::D_IN].astype(np.int64)          # source node of each edge
    kidx = (row[:, None] * D_IN + np.arange(D_IN)[None, :]).reshape(-1)  # [T]
    structured = (
        np.array_equal(
            e_e, np.repeat(np.arange(E, dtype=np.int64), D_IN).astype(e_e.dtype)
        )
        and np.array_equal(j_e.astype(np.int64), e_e.astype(np.int64) // D_IN)
        and np.array_equal(i_e.astype(np.int64), np.repeat(row, D_IN))
        and np.array_equal(k_e.astype(np.int64), row[kidx])
    )
    if not structured:
        return _numpy_fallback(pos, W1, b1, W2, b2, rc, ac, e_e, i_e, j_e, k_e)

    # Per-edge geometry on host (E values instead of T), then expand to
    # triplets; device handles RBF + MLP + segment sum.
    col = np.repeat(np.arange(N, dtype=np.int64), D_IN)
    dvec = pos[col] - pos[row]                  # [E,3]
    d = np.sqrt((dvec * dvec).sum(-1))          # [E]
    u = dvec / np.maximum(d, 1e-30)[:, None]    # [E,3] unit vectors

    dik = d[kidx]                               # [T]
    # edge kidx points k->i, so rik = pos[k]-pos[i] = -dvec[kidx]
    cos = np.clip(
        -np.einsum("ts,ts->t", np.repeat(u, D_IN, axis=0), u[kidx]), -1.0, 1.0
    )
    mask = k_e != j_e

    xd = d.astype(np.float16).reshape(1, E)
    xkc = np.empty((2, T), np.float16)
    xkc[0] = np.where(mask, dik, POISON)
    xkc[1] = np.where(mask, cos, POISON)

    # Feature f <- source row s(f), center c_f:
    #   exp(-g*(x-c)^2) = exp(-g*x^2 + 2*g*c*x - g*c^2)
    cf = np.concatenate([rc, rc, ac]).astype(np.float32)         # [40]
    prm = np.zeros(P_TOT, np.float32)
    prm[P_KDA : P_KDA + K_R] = 2.0 * GAMMA * rc
    prm[P_KDB : P_KDB + K_R] = -GAMMA
    ka2 = np.zeros((2, IN_DIM), np.float32)
    ka2[0, K_R : 2 * K_R] = 2.0 * GAMMA * rc
    ka2[1, 2 * K_R :] = 2.0 * GAMMA * ac
    kb2 = np.zeros((2, IN_DIM), np.float32)
    kb2[0, K_R : 2 * K_R] = -GAMMA
    kb2[1, 2 * K_R :] = -GAMMA
    prm[P_KA2 : P_KA2 + 80] = ka2.reshape(-1)
    prm[P_KB2 : P_KB2 + 80] = kb2.reshape(-1)
    prm[P_BIAS : P_BIAS + 40] = -GAMMA * cf * cf
    prm[P_W1 : P_W1 + 2560] = W1.reshape(-1)
    prm[P_B1 : P_B1 + 64] = b1
    prm[P_W2 : P_W2 + 2048] = W2.reshape(-1)

    in_maps = []
    for dev in range(NCORES):
        in_maps.append(
            {
                "xd": np.ascontiguousarray(xd[:, dev * ED : (dev + 1) * ED]),
                "xkc": np.ascontiguousarray(xkc[:, dev * TD : (dev + 1) * TD]),
                "params": prm,
            }
        )

    import time as _time

    global LAST_RUN_S
    _t0 = _time.time()
    res = run_bass_kernel_spmd(_get_program(), in_maps, list(range(NCORES)))
    LAST_RUN_S = _time.time() - _t0
    LAST_RESULTS = res
    outT = np.concatenate([res.results[dev]["y"] for dev in range(NCORES)], axis=1)
    out = np.ascontiguousarray(outT.T.astype(np.float32))

    # Masked (k==j) triplets: xd is per-edge so the dij RBF block couldn't be
    # poisoned on device; those triplets contributed silu(W1a^T f_ij + b1)@W2.
    # Subtract that contribution exactly (few hundred triplets).
    t_bad = np.nonzero(~mask)[0]
    if t_bad.size:
        e_bad = t_bad // D_IN
        d_bad = xd[0, e_bad].astype(np.float32)
        f_ij = np.exp(-GAMMA * (d_bad[:, None] - rc[None, :]) ** 2)
        hpre = f_ij @ W1[:K_R] + b1
        hb = hpre / (1.0 + np.exp(-hpre))
        np.subtract.at(out, e_bad, (hb @ W2).astype(np.float32))

    if b2.any():
        cnt = np.bincount(e_e, weights=mask.astype(np.float64), minlength=E)
        out = out + cnt[:, None].astype(np.float32) * b2[None, :]
    return out
